# revision 38
# baseline (speedup 1.0000x reference)
"""KNN-Attention Trainium2 kernel.

Sharding: 8 cores = 4 batches x 2 head-groups (8 heads each).
Each core computes a partial output [T, E] = combined_slice @ W_proj_slice;
host sums the two partials per batch.

Per-core device program (SPMD, all per-core variation flows through inputs):
  - fp32r matmuls (1 cyc/row at free>=256, ~12-bit mantissa) for all
    projections; q computed once, naturally, then PE-transposed for qT
  - mem attention: fp32 scores on DVE (q precision bounds slot-argmax
    flips), K=3 softmax; value blend muls on DVE, adds on Pool; mem_v/mm
    bf16 with g prescaled (value precision can't flip slot selection)
  - main attention per head: S^T [128tk, <=512tq] bf16 with causal
    narrowing, causal mask folded into PSUM via an identity-stationary
    matmul adding -1e5 above the diagonal (no elementwise mask, no
    cross-engine dependency); one exp per pair of key tiles on ACT;
    AV flipped (lhsT=pt block, moving=v [128k,65]) -> y natural
    [128q,65] with denominators in col 64, software-pipelined LAG pairs
    behind S. PSUM has_written is cleared bank-wide by start=True, so
    only the first AV matmul into the y tile uses start; single stop.
  - combine: scalar_tensor_tensor fuses y*(1/den)+mem (per-partition scalar)
  - schedule: "mix" sections (proj(c) on PE with PSUM-release copies on
    ACT, mem(c) on DVE/Pool) alternate with attn(c) sections that carry
    woven cproj(c-1), late kT/qT blocks, and (for the last chunk) per-fb
    comb transposes, so every engine runs disjoint work and PE keeps
    matmul work queued during ACT's exp stream.
"""

import numpy as np
import ml_dtypes

import concourse.bass as bass
import concourse.mybir as mybir
import concourse.tile as tile
from concourse import bacc
from concourse.bass_utils import run_bass_kernel_spmd
from concourse.masks import make_identity

B, T, E, H, KSLOT = 4, 2048, 1024, 16, 3
D = E // H          # 64
HPC = 8             # heads per core
EC = HPC * D        # 512 cols per core
NCORES = 8
TC = 512            # t-chunk
NCHUNK = T // TC    # 4

f32 = mybir.dt.float32
f32r = mybir.dt.float32r
bf16 = mybir.dt.bfloat16

_CACHE = {}


def _build_nc():
    nc = bacc.Bacc("TRN2", target_bir_lowering=False, debug=False)

    # ---- DRAM I/O ----
    xTf = nc.dram_tensor("xTf", [E, T], f32r, kind="ExternalInput").ap()
    wqkv = nc.dram_tensor("wqkv", [E, 3 * EC], f32r, kind="ExternalInput").ap()
    wp = nc.dram_tensor("wp", [EC, E], bf16, kind="ExternalInput").ap()
    mk = nc.dram_tensor("mk", [T, KSLOT * EC], f32, kind="ExternalInput").ap()
    mvg = nc.dram_tensor("mvg", [T, KSLOT * EC], bf16, kind="ExternalInput").ap()
    mneg = nc.dram_tensor("mneg", [128, 128], bf16, kind="ExternalInput").ap()
    out = nc.dram_tensor("out", [T, E], f32, kind="ExternalOutput").ap()

    xTf_r = xTf.rearrange("(ko p) t -> p ko t", p=128)      # [128, 8, T]
    wqkv_r = wqkv.rearrange("(ko p) n -> p ko n", p=128)    # [128, 8, 1536]
    wp_r = wp.rearrange("(ko p) n -> p ko n", p=128)        # [128, 4, 1024]

    mem_scale = float(E) * float(np.sqrt(H))

    with tile.TileContext(nc) as tc:
        with (
            tc.tile_pool(name="consts", bufs=1) as consts,
            tc.tile_pool(name="xtf", bufs=2) as xtfp,
            tc.tile_pool(name="qn", bufs=1) as qnp,
            tc.tile_pool(name="qt", bufs=2) as qtp,
            tc.tile_pool(name="ptl", bufs=1) as ptp,
            tc.tile_pool(name="comb", bufs=1) as combp,
            tc.tile_pool(name="mkp", bufs=1) as mkp,
            tc.tile_pool(name="mvp", bufs=2) as mvp,
            tc.tile_pool(name="prodp", bufs=2) as prodp,
            tc.tile_pool(name="mm", bufs=1) as mmp,
            tc.tile_pool(name="small", bufs=2) as small,
            tc.tile_pool(name="ost", bufs=3) as ostp,
            tc.tile_pool(name="pp", bufs=2, space="PSUM") as pp,
            tc.tile_pool(name="sps", bufs=1, space="PSUM") as spsp,
            tc.tile_pool(name="yp", bufs=2, space="PSUM") as ypp,
        ):
            # ---- persistent SBUF ----
            wqkv_sb = consts.tile([128, 8, 3 * EC], f32r, tag="wqkv")
            wp_sb = consts.tile([128, 4, E], bf16, tag="wp")
            mneg_sb = consts.tile([128, 128], bf16, tag="mneg")
            identf = consts.tile([128, 128], f32, tag="identf")
            identb = consts.tile([128, 128], bf16, tag="identb")
            kT_sb = consts.tile([128, 4, T], bf16, tag="kT")
            v_sb = consts.tile([128, T // 128, HPC, D + 1], bf16, tag="v")

            st = {}  # per-chunk tile handles

            def emit_consts_and_first_dmas():
                # first-chunk x and q-weights first so qnat can start early
                xtf_c = xtfp.tile([128, 8, TC], f32r, tag="xtf")
                st[("xtf", 0)] = xtf_c
                nc.sync.dma_start(xtf_c[:], xTf_r[:, :, 0:TC])
                nc.scalar.dma_start(wqkv_sb[:, :, 0:EC], wqkv_r[:, :, 0:EC])
                for ke in range(8):
                    nc.scalar.dma_start(
                        wqkv_sb[:, ke, EC : 2 * EC], wqkv_r[:, ke, EC : 2 * EC]
                    )
                for ke in range(8):
                    nc.scalar.dma_start(
                        wqkv_sb[:, ke, 2 * EC : 3 * EC], wqkv_r[:, ke, 2 * EC : 3 * EC]
                    )
                nc.scalar.dma_start(wp_sb[:], wp_r)
                nc.scalar.dma_start(mneg_sb[:], mneg)
                make_identity(nc, identf[:])
                nc.vector.tensor_copy(identb[:], identf[:])
                nc.vector.memset(v_sb[:, :, :, D], 1.0)

            def emit_proj(c, mem_cb=None):
                ts = slice(c * TC, (c + 1) * TC)
                if ("xtf", c) not in st:
                    xtf_c = xtfp.tile([128, 8, TC], f32r, tag="xtf")
                    st[("xtf", c)] = xtf_c
                    nc.sync.dma_start(xtf_c[:], xTf_r[:, :, ts])
                xtf_c = st[("xtf", c)]

                # qnat: q natural [tb*128, 512] fp32r
                qn_c = qnp.tile([128, 4, TC], f32, tag="qn")
                st[("qn", c)] = qn_c
                for tb in range(4):
                    ps = pp.tile([128, TC], f32, tag="pp512")
                    for ke in range(8):
                        nc.tensor.matmul(
                            ps[:],
                            xtf_c[:, ke, 128 * tb : 128 * (tb + 1)],
                            wqkv_sb[:, ke, 0:EC],
                            start=(ke == 0),
                            stop=(ke == 7),
                        )
                    cpy(qn_c[:, tb, :], ps[:])

                if mem_cb is not None:
                    mem_cb(c - 1, 0)

                # qT: PE-transpose qnat (f32), copy to bf16
                qT_c = qtp.tile([128, 4, TC], bf16, tag="qT")
                st[("qT", c)] = qT_c
                for fb in range(4):
                    tp = pp.tile([128, TC], f32, tag="pp512")
                    for tb in range(4):
                        nc.tensor.transpose(
                            tp[:, 128 * tb : 128 * (tb + 1)],
                            qn_c[:, tb, 128 * fb : 128 * (fb + 1)],
                            identf[:],
                        )
                    cpy(qT_c[:, fb, :], tp[:])

                if mem_cb is not None:
                    mem_cb(c - 1, 1)

                # kT: [512, T] fp32r, transposed layout
                for m in range(4):
                    ps = pp.tile([128, TC], f32, tag="pp512")
                    for ke in range(8):
                        nc.tensor.matmul(
                            ps[:],
                            wqkv_sb[:, ke, EC + 128 * m : EC + 128 * (m + 1)],
                            xtf_c[:, ke, :],
                            start=(ke == 0),
                            stop=(ke == 7),
                        )
                    cpy(kT_sb[:, m, ts], ps[:])

                if mem_cb is not None:
                    mem_cb(c - 1, 2)

                # v natural [tb*128, 512] fp32r -> bf16 (+ones col kept)
                for tb in range(4):
                    ps = pp.tile([128, TC], f32, tag="pp512")
                    for ke in range(8):
                        nc.tensor.matmul(
                            ps[:],
                            xtf_c[:, ke, 128 * tb : 128 * (tb + 1)],
                            wqkv_sb[:, ke, 2 * EC : 3 * EC],
                            start=(ke == 0),
                            stop=(ke == 7),
                        )
                    cpy(
                        v_sb[:, 4 * c + tb, :, 0:D],
                        ps[:].rearrange("p (h d) -> p h d", d=D),
                    )
                if mem_cb is not None:
                    mem_cb(c - 1, 3)

                # prefetch next chunk's x
                if c + 1 < NCHUNK:
                    nxt = slice((c + 1) * TC, (c + 2) * TC)
                    xtf_n = xtfp.tile([128, 8, TC], f32r, tag="xtf")
                    st[("xtf", c + 1)] = xtf_n
                    nc.sync.dma_start(xtf_n[:], xTf_r[:, :, nxt])

            def emit_mem_score(c, tb, mul_on_dve=False):
                qn_c = st[("qn", c)]
                if True:
                    trow = slice(c * TC + 128 * tb, c * TC + 128 * (tb + 1))
                    mk_t = mkp.tile([128, KSLOT, EC], f32, tag="mk")
                    nc.gpsimd.dma_start(
                        mk_t[:], mk[trow, :].rearrange("p (k e) -> p k e", k=KSLOT)
                    )
                    mv_t = mvp.tile([128, KSLOT, EC], bf16, tag="mv")
                    nc.gpsimd.dma_start(
                        mv_t[:], mvg[trow, :].rearrange("p (k e) -> p k e", k=KSLOT)
                    )

                    # exact fp32 scores: s3[t, k, h] = sum_d qn*mk
                    prod = prodp.tile([128, KSLOT, EC], f32, tag="prod")
                    nc.vector.tensor_mul(
                        prod[:],
                        mk_t[:],
                        qn_c[:, tb, None, :].to_broadcast((128, KSLOT, EC)),
                    )
                    s3 = small.tile([128, KSLOT, HPC], f32, tag="s3")
                    nc.vector.tensor_reduce(
                        s3[:],
                        prod[:].rearrange("p k (h d) -> p k h d", d=D),
                        mybir.AxisListType.X,
                        mybir.AluOpType.add,
                    )
                    m3 = small.tile([128, HPC], f32, tag="m3")
                    nc.vector.tensor_reduce(
                        m3[:],
                        s3[:].rearrange("p k h -> p h k"),
                        mybir.AxisListType.X,
                        mybir.AluOpType.max,
                    )
                    z3 = small.tile([128, KSLOT, HPC], f32, tag="z3")
                    nc.vector.tensor_sub(
                        z3[:], s3[:], m3[:, None, :].to_broadcast((128, KSLOT, HPC))
                    )
                    e3 = small.tile([128, KSLOT, HPC], f32, tag="e3")
                    nc.scalar.activation(
                        e3[:], z3[:], mybir.ActivationFunctionType.Exp,
                        scale=mem_scale,
                    )
                    den = small.tile([128, HPC], f32, tag="den")
                    nc.vector.tensor_reduce(
                        den[:],
                        e3[:].rearrange("p k h -> p h k"),
                        mybir.AxisListType.X,
                        mybir.AluOpType.add,
                    )
                    rden = small.tile([128, HPC], f32, tag="rden")
                    nc.vector.reciprocal(rden[:], den[:])
                    w3 = small.tile([128, KSLOT, HPC], f32, tag="w3")
                    nc.vector.tensor_mul(
                        w3[:], e3[:], rden[:, None, :].to_broadcast((128, KSLOT, HPC))
                    )
                    # value blend: wprod[t,k,e] = w3[t,k,h(e)]*mvg[t,k,e]
                    wprod = prodp.tile([128, KSLOT, EC], f32, tag="prod")
                    eng = nc.vector if mul_on_dve else nc.gpsimd
                    eng.tensor_mul(
                        wprod[:].rearrange("p k (h d) -> p k h d", d=D),
                        mv_t[:].rearrange("p k (h d) -> p k h d", d=D),
                        w3[:, :, :, None].to_broadcast((128, KSLOT, HPC, D)),
                    )
                    st[("wprod", c, tb)] = wprod

            def emit_mem_reduce(c, tb):
                # mm[t,e] = sum_k wprod[t,k,e] on Pool
                if ("mm", c) not in st:
                    mm_c = mmp.tile([128, 4, EC], bf16, tag="mm")
                    st[("mm", c)] = mm_c
                mm_c = st[("mm", c)]
                wprod = st.pop(("wprod", c, tb))
                nc.gpsimd.tensor_add(
                    mm_c[:, tb, :], wprod[:, 0, :], wprod[:, 1, :]
                )
                nc.gpsimd.tensor_add(
                    mm_c[:, tb, :], mm_c[:, tb, :], wprod[:, 2, :]
                )

            def emit_mem(c):
                for tb in range(4):
                    emit_mem_score(c, tb)
                    emit_mem_reduce(c, tb)

            def emit_attn(c):
                qT_c = st[("qT", c)]
                mm_c = st[("mm", c)]
                njt = 4 * c + 4
                npairs = njt // 2
                LAG = 3
                pt_c = ptp.tile([128, 16, TC], bf16, tag="pt")
                comb_c = combp.tile([128, 4, EC], f32, tag="comb")
                st[("comb", c)] = comb_c
                YW = 96
                for h in range(HPC):
                    prow = slice(64 * (h % 2), 64 * (h % 2) + 64)
                    pc = h // 2
                    y_h = ypp.tile([128, 4, YW], f32, tag="y")
                    for p in range(npairs + LAG):
                        if p < npairs:
                            j0 = 2 * p
                            offp = 128 * (j0 - 4 * c) if j0 >= 4 * c else 0
                            sps = spsp.tile([128, 2, TC], f32, tag="sps")
                            for dj in range(2):
                                j = j0 + dj
                                diag = j >= 4 * c
                                off = 128 * (j - 4 * c) if diag else 0
                                nc.tensor.matmul(
                                    sps[:, dj, off:TC],
                                    kT_sb[prow, pc, 128 * j : 128 * (j + 1)],
                                    qT_c[prow, pc, off:TC],
                                    start=True,
                                    stop=not diag,
                                )
                                if diag:
                                    # causal mask: add -1e5 above the diagonal
                                    # (identity-stationary injects mneg rows)
                                    nc.tensor.matmul(
                                        sps[:, dj, off : off + 128],
                                        identb[:],
                                        mneg_sb[:],
                                        start=False,
                                        stop=True,
                                        skip_group_check=True,
                                    )
                            # one exp per pair (junk cols of the narrower
                            # member hold stale finite scores; never read)
                            nc.scalar.activation(
                                pt_c[:, j0 : j0 + 2, offp:TC],
                                sps[:, :, offp:TC],
                                mybir.ActivationFunctionType.Exp,
                                scale=1.0 / float(np.sqrt(D)),
                            )
                        # AV for pair p-LAG. PSUM has_written is cleared
                        # bank-wide by start=True: only the first matmul into
                        # the tile starts; later first-writes to other regions
                        # overwrite-on-cleared-bit; single stop at the end.
                        if p >= LAG:
                            for j in range(2 * (p - LAG), min(2 * (p - LAG) + 2, njt)):
                                rmin = j - 4 * c if j >= 4 * c else 0
                                for r in range(rmin, 4):
                                    nc.tensor.matmul(
                                        y_h[:, r, 0 : D + 1],
                                        pt_c[:, j, 128 * r : 128 * (r + 1)],
                                        v_sb[:, j, h, :],
                                        start=(j == 0 and r == 0),
                                        stop=(j == njt - 1 and r == 3),
                                        skip_group_check=True,
                                    )
                    # normalize + combine with mem output
                    rg = small.tile([128, 4], f32, tag="rg")
                    nc.vector.reciprocal(rg[:], y_h[:, :, D])
                    for r in range(4):
                        nc.vector.scalar_tensor_tensor(
                            comb_c[:, r, D * h : D * (h + 1)],
                            y_h[:, r, 0:D],
                            rg[:, r : r + 1],
                            mm_c[:, r, D * h : D * (h + 1)],
                            mybir.AluOpType.mult,
                            mybir.AluOpType.add,
                        )

            def emit_cproj(c):
                comb_c = st[("comb", c)]
                # comb -> combT (f32 PE transposes via pp pool)
                combT_c = qtp.tile([128, 4, EC], bf16, tag="combT")
                for fb in range(4):
                    tp = pp.tile([128, TC], f32, tag="pp512")
                    for r in range(4):
                        nc.tensor.transpose(
                            tp[:, 128 * r : 128 * (r + 1)],
                            comb_c[:, r, 128 * fb : 128 * (fb + 1)],
                            identf[:],
                        )
                    nc.vector.tensor_copy(combT_c[:, fb, :], tp[:])

                for tb in range(4):
                    trow = slice(c * TC + 128 * tb, c * TC + 128 * (tb + 1))
                    for n in range(2):
                        ps = pp.tile([128, TC], f32, tag="pp512")
                        for ke in range(4):
                            nc.tensor.matmul(
                                ps[:],
                                combT_c[:, ke, 128 * tb : 128 * (tb + 1)],
                                wp_sb[:, ke, TC * n : TC * (n + 1)],
                                start=(ke == 0),
                                stop=(ke == 3),
                            )
                        ost = ostp.tile([128, TC], f32, tag="ost")
                        if n == 0:
                            nc.scalar.copy(ost[:], ps[:])
                        else:
                            nc.vector.tensor_copy(ost[:], ps[:])
                        nc.sync.dma_start(out[trow, TC * n : TC * (n + 1)], ost[:])

            def emit_mix(c):
                """proj(c) on PE with PSUM copies on ACT, mem(c) on DVE/Pool,
                interleaved so all four engines run disjoint work."""
                for tb in range(4):
                    emit_qnat_tb(c, tb, on_act=True)
                emit_qT(c, on_act=True)
                emit_mem_score(c, 0, mul_on_dve=True)
                emit_kT_m(c, 0, on_act=True)
                emit_kT_m(c, 1, on_act=True)
                emit_mem_reduce(c, 0)
                emit_mem_score(c, 1, mul_on_dve=True)
                emit_kT_m(c, 2, on_act=True)
                emit_kT_m(c, 3, on_act=True)
                emit_mem_reduce(c, 1)
                emit_mem_score(c, 2, mul_on_dve=True)
                emit_v_tb(c, 0, on_act=True)
                emit_v_tb(c, 1, on_act=True)
                emit_mem_reduce(c, 2)
                emit_mem_score(c, 3, mul_on_dve=True)
                emit_v_tb(c, 2, on_act=True)
                emit_v_tb(c, 3, on_act=True)
                emit_mem_reduce(c, 3)
                emit_xtf_dma(c + 1)

            # ---- chunk software pipeline ----
            # [proj+mem mix(c)] | [attn(c) with cproj(c-1) woven] — the mix
            # sections keep PE/ACT/DVE/Pool on disjoint work; attention's
            # idle PE slots host the previous chunk's output projection.
            emit_consts_and_first_dmas()
            emit_mix(0)
            for c in range(NCHUNK):
                weave = {h: [] for h in range(HPC)}
                if c >= 1:
                    weave[0] += [lambda: emit_combT(c - 1)]
                    for tb in range(4):
                        weave[2 * tb] += [lambda tb=tb: emit_cproj_tb(c - 1, tb)]
                emit_attn(c, weave=weave)
                if c + 1 < NCHUNK:
                    emit_mix(c + 1)
            emit_cproj(NCHUNK - 1)

    nc.compile()
    return nc


def _prep_inputs(x, mem_k, mem_v, W_attn, W_proj, gate_bias):
    """Build per-core input maps (host-side sharding/layout only)."""
    in_maps = []
    g = gate_bias.reshape(H)
    # mneg[k, q] = -1e5 where k > q (strictly below diagonal = masked)
    mneg = -1.0e5 * np.tril(np.ones((128, 128), dtype=np.float32), k=-1)
    for core in range(NCORES):
        b, hg = core // 2, core % 2
        cs = slice(hg * EC, (hg + 1) * EC)
        xb = np.asarray(x[b], dtype=np.float32)            # [T, E]
        xT = np.ascontiguousarray(xb.T)                    # [E, T]
        gh = g[hg * HPC : (hg + 1) * HPC].astype(np.float32)   # [8]
        wq = np.ascontiguousarray(W_attn[:, cs])           # [E, 512]
        wk = np.ascontiguousarray(W_attn[:, E + hg * EC : E + (hg + 1) * EC])
        wv = np.ascontiguousarray(W_attn[:, 2 * E + hg * EC : 2 * E + (hg + 1) * EC])
        # fold (1-g) into W_v: y uses v*(1-g)
        wv = wv * (1.0 - gh).repeat(D)[None, :]
        wqkv = np.concatenate([wq, wk, wv], axis=1)        # [E, 1536]
        mkc = np.ascontiguousarray(mem_k[b][:, :, cs]).reshape(T, KSLOT * EC)
        mvc = np.ascontiguousarray(mem_v[b][:, :, cs]).astype(np.float32)
        # fold gate into mem_v: combined = mem*g + y*(1-g)
        mvc = mvc * gh.repeat(D)[None, None, :]
        mvc = mvc.reshape(T, KSLOT * EC)
        wpc = np.ascontiguousarray(W_proj[cs, :])          # [512, E]
        in_maps.append(
            {
                "xTf": xT,
                "wqkv": wqkv.astype(np.float32),
                "wp": wpc.astype(ml_dtypes.bfloat16),
                "mk": mkc.astype(np.float32),
                "mvg": mvc.astype(ml_dtypes.bfloat16),
                "mneg": mneg.astype(ml_dtypes.bfloat16),
            }
        )
    return in_maps


def kernel(x, mem_k, mem_v, W_attn, W_proj, gate_bias, **kw):
    x = np.asarray(x, dtype=np.float32)
    mem_k = np.asarray(mem_k, dtype=np.float32)
    mem_v = np.asarray(mem_v, dtype=np.float32)
    W_attn = np.asarray(W_attn, dtype=np.float32)
    W_proj = np.asarray(W_proj, dtype=np.float32)
    gate_bias = np.asarray(gate_bias, dtype=np.float32)

    if "nc" not in _CACHE:
        _CACHE["nc"] = _build_nc()
    nc = _CACHE["nc"]
    in_maps = _prep_inputs(x, mem_k, mem_v, W_attn, W_proj, gate_bias)
    res = run_bass_kernel_spmd(nc, in_maps, list(range(NCORES)), **kw)
    results = res.results if hasattr(res, "results") else res
    out = np.empty((B, T, E), dtype=np.float32)
    for b in range(B):
        out[b] = results[2 * b]["out"] + results[2 * b + 1]["out"]
    _CACHE["last_res"] = res
    return out
